# revision 28
# baseline (speedup 1.0000x reference)
"""Trainium2 Bass kernel for nn_LAMME (conv3x3 + LAM temporal attention + ME gate).

Data-parallel over 8 NeuronCores: each core processes one clip of t=8 frames
(c=256, h=w=56).  Single fused kernel per core.

The 3x3 conv uses a 1-D Winograd F(2,3) transform along the ROW (height)
axis: input rows are split host-side into even/odd planes E/O of the 58x58
zero-padded frame; DVE computes 4 Winograd planes per row-tile r
  d0=E[r]-E[r+1], d1=O[r]+E[r+1], d2=E[r+1]-O[r], d3=O[r]-O[r+1]
and the PE contracts them against G-transformed weights (per dx shift), so
the PE streams 2/3 of the columns of a direct conv.  Two output rows come
back via e=m0+m1+m2, o=m1-m2-m3 (DVE adds on bf16 copies of PSUM).

Pooled means (which feed the LAM softmax weights and ME sigmoid gates) are
computed analytically from per-frame window sums of x (row/col/total
reductions of the resident xin tiles), so all gates are known ~60us in and
phase 2 (temporal conv + gating + f32 output DMA) streams 2 frames behind
the conv instead of serializing after it.
"""
import sys
for p in ('/opt/trn_rl_repo',):
    if p not in sys.path:
        sys.path.insert(0, p)

import numpy as np
import ml_dtypes

import concourse.bacc as bacc
import concourse.mybir as mybir
import concourse.tile as tile
from concourse.bass_utils import run_bass_kernel_spmd

F32 = mybir.dt.float32
BF16 = mybir.dt.bfloat16
AF = mybir.ActivationFunctionType
OP = mybir.AluOpType
AX = mybir.AxisListType

T = 8          # frames per clip (= clips per core after sharding)
NCORES = 8
HP = 58        # padded spatial width
PADSZ = HP * HP  # 3364
NT = 28        # winograd row-tiles per frame (56 out rows / 2)
NB = 4         # row-tile blocks per frame (8+8+8+4)
BLK = [(0, 8), (8, 8), (16, 8), (24, 4)]   # (tile0, ntiles) per block

_CACHE = {}


def _build(me):
    nc = bacc.Bacc("TRN2", target_bir_lowering=False, debug=False)

    # x layout per frame: [2ci_t, 128, 2eo, 29, 58] (even/odd padded rows)
    x_d = nc.dram_tensor("x", [T, 2, 128, PADSZ], BF16, kind="ExternalInput")
    gw_d = nc.dram_tensor("gw", [48, 128, 128], BF16, kind="ExternalInput")
    lamw_d = nc.dram_tensor("lamw", [4, 128, 128], BF16, kind="ExternalInput")
    w1t_d = nc.dram_tensor("w1t", [8, 16], BF16, kind="ExternalInput")
    w2t_d = nc.dram_tensor("w2t", [16, 3], BF16, kind="ExternalInput")
    bns_d = nc.dram_tensor("bns", [16, 1], F32, kind="ExternalInput")
    bnsh_d = nc.dram_tensor("bnsh", [16, 1], F32, kind="ExternalInput")
    netb_d = nc.dram_tensor("netb", [2, 128], F32, kind="ExternalInput")
    lamb_d = nc.dram_tensor("lamb", [2, 128], F32, kind="ExternalInput")
    id_d = nc.dram_tensor("ident", [128, 128], BF16, kind="ExternalInput")
    out_d = nc.dram_tensor("out", [T, 256, 3136], F32, kind="ExternalOutput")

    def gidx(ci_t, j, dx, co_t):
        return (((ci_t * 4 + j) * 3 + dx) * 2 + co_t)

    with tile.TileContext(nc) as tc:
        with (
            tc.tile_pool(name="const", bufs=1) as cpool,
            tc.tile_pool(name="xp", bufs=4) as xpool,
            tc.tile_pool(name="dt", bufs=4) as dpool,
            tc.tile_pool(name="raw", bufs=4) as rawpool,
            tc.tile_pool(name="cp", bufs=2) as cppool,
            tc.tile_pool(name="work", bufs=2) as wpool,
            tc.tile_pool(name="fin", bufs=2) as fpool,
            tc.tile_pool(name="small", bufs=1) as spool,
            tc.tile_pool(name="mpsum", bufs=2, space="PSUM") as mpsum,
        ):
            # frame 0 split into two row-range tiles (rows 0-16 / 16-28 of
            # the E/O planes, row 16 duplicated) so the first transforms and
            # matmuls only wait on the first ~1MB of DMA
            xin_t = {}
            xin0a = xpool.tile([128, 2, 2, 17, HP], BF16, tag="xin", name="xin")
            xin0b = xpool.tile([128, 2, 2, 13, HP], BF16, tag="xin", name="xin")
            for ci in range(2):
                xv0 = x_d.ap()[0, ci].rearrange("p (e r x) -> p e r x", e=2, x=HP)
                for eo in range(2):
                    nc.sync.dma_start(out=xin0a[:, ci, eo], in_=xv0[:, eo, 0:17, :])
            for ci in range(2):
                xv0 = x_d.ap()[0, ci].rearrange("p (e r x) -> p e r x", e=2, x=HP)
                for eo in range(2):
                    nc.sync.dma_start(out=xin0b[:, ci, eo], in_=xv0[:, eo, 16:29, :])
            gw_sb = cpool.tile([128, 48, 128], BF16)
            for h in range(4):
                nc.sync.dma_start(
                    out=gw_sb[:, h * 12:(h + 1) * 12],
                    in_=gw_d.ap()[h * 12:(h + 1) * 12].rearrange("w p m -> p w m"))
            lamw_sb = cpool.tile([128, 4, 128], BF16)
            nc.sync.dma_start(out=lamw_sb[:], in_=lamw_d.ap().rearrange("w p m -> p w m"))
            w1t_sb = cpool.tile([8, 16], BF16)
            nc.sync.dma_start(out=w1t_sb[:], in_=w1t_d.ap())
            w2t_sb = cpool.tile([16, 3], BF16)
            nc.sync.dma_start(out=w2t_sb[:], in_=w2t_d.ap())
            bns_sb = cpool.tile([16, 1], F32)
            nc.sync.dma_start(out=bns_sb[:], in_=bns_d.ap())
            bnsh_sb = cpool.tile([16, 1], F32)
            nc.sync.dma_start(out=bnsh_sb[:], in_=bnsh_d.ap())
            netb_sb = cpool.tile([128, 2], F32)
            nc.sync.dma_start(out=netb_sb[:], in_=netb_d.ap().rearrange("t p -> p t"))
            lamb_sb = cpool.tile([128, 2], F32)
            nc.sync.dma_start(out=lamb_sb[:], in_=lamb_d.ap().rearrange("t p -> p t"))
            id_sb = cpool.tile([128, 128], BF16)
            nc.sync.dma_start(out=id_sb[:], in_=id_d.ap())

            # stats accumulators
            acc_tot = spool.tile([128, 2, NB, T], F32)  # per-(ci,block) d1 sums
            st_tot = spool.tile([128, 2, T], F32)
            st_r1 = spool.tile([128, 2, T], F32)
            st_r56 = spool.tile([128, 2, T], F32)
            st_c1 = spool.tile([128, 2, T], F32)
            st_c56 = spool.tile([128, 2, T], F32)
            st_cA = spool.tile([128, 2, 2, T], F32)   # row-1 corners (1,1),(1,56)
            st_cB = spool.tile([128, 2, 2, T], F32)   # row-56 corners (56,1),(56,56)

            def emit_xin_dma(f):
                xin = xpool.tile([128, 2, PADSZ], BF16, tag="xin", name="xin")
                xin_t[f] = xin
                nc.sync.dma_start(
                    out=xin[:], in_=x_d.ap()[f].rearrange("t p m -> p t m"))

            def emit_stats(f):
                if f == 0:
                    # split-tile variant; total comes from the accum path
                    sa, sb = xin0a, xin0b
                    nc.vector.tensor_reduce(
                        out=st_r1[:, :, f], in_=sa[:, :, 1, 0, :], axis=AX.X, op=OP.add)
                    nc.vector.tensor_reduce(
                        out=st_r56[:, :, f], in_=sb[:, :, 0, 12, :], axis=AX.X, op=OP.add)
                    tmpc = spool.tile([128, 2, 2], F32)
                    nc.vector.tensor_reduce(
                        out=tmpc[:, :, 0], in_=sa[:, :, :, 0:16, 1], axis=AX.XY, op=OP.add)
                    nc.vector.tensor_reduce(
                        out=tmpc[:, :, 1], in_=sb[:, :, :, :, 1], axis=AX.XY, op=OP.add)
                    nc.vector.tensor_reduce(
                        out=st_c1[:, :, f], in_=tmpc[:], axis=AX.X, op=OP.add)
                    nc.vector.tensor_reduce(
                        out=tmpc[:, :, 0], in_=sa[:, :, :, 0:16, 56], axis=AX.XY, op=OP.add)
                    nc.vector.tensor_reduce(
                        out=tmpc[:, :, 1], in_=sb[:, :, :, :, 56], axis=AX.XY, op=OP.add)
                    nc.vector.tensor_reduce(
                        out=st_c56[:, :, f], in_=tmpc[:], axis=AX.X, op=OP.add)
                    nc.vector.tensor_copy(
                        out=st_cA[:, :, :, f], in_=sa[:, :, 1, 0, 1::55])
                    nc.vector.tensor_copy(
                        out=st_cB[:, :, :, f], in_=sb[:, :, 0, 12, 1::55])
                    return
                xin = xin_t[f]
                srv = xin.rearrange("p c (e r x) -> p c e r x", e=2, x=HP)
                if f >= 5:
                    # frames whose transforms are emitted after emit_small():
                    # the accum_out trick can't supply the total, reduce here
                    nc.vector.tensor_reduce(
                        out=st_tot[:, :, f], in_=xin[:], axis=AX.X, op=OP.add)
                nc.vector.tensor_reduce(
                    out=st_r1[:, :, f], in_=srv[:, :, 1, 0, :], axis=AX.X, op=OP.add)
                nc.vector.tensor_reduce(
                    out=st_r56[:, :, f], in_=srv[:, :, 0, 28, :], axis=AX.X, op=OP.add)
                nc.vector.tensor_reduce(
                    out=st_c1[:, :, f], in_=srv[:, :, :, :, 1], axis=AX.XY, op=OP.add)
                nc.vector.tensor_reduce(
                    out=st_c56[:, :, f], in_=srv[:, :, :, :, 56], axis=AX.XY, op=OP.add)
                nc.vector.tensor_copy(
                    out=st_cA[:, :, :, f], in_=srv[:, :, 1, 0, 1::55])
                nc.vector.tensor_copy(
                    out=st_cB[:, :, :, f], in_=srv[:, :, 0, 28, 1::55])

            dt_t = {}

            def emit_transform(f, b):
                """winograd planes for row-tile block b of frame f"""
                r0, nt = BLK[b]
                if f == 0:
                    srv = xin0a if b < 2 else xin0b
                    rb = r0 if b < 2 else r0 - 16
                else:
                    srv = xin_t[f].rearrange("p c (e r x) -> p c e r x", e=2, x=HP)
                    rb = r0
                dt = dpool.tile([128, 2, 4, 8, HP], BF16, tag="dt", name="dt")
                dt_t[(f, b)] = dt
                E0 = srv[:, :, 0, rb:rb + nt, :]
                E1 = srv[:, :, 0, rb + 1:rb + nt + 1, :]
                O0 = srv[:, :, 1, rb:rb + nt, :]
                O1 = srv[:, :, 1, rb + 1:rb + nt + 1, :]
                nc.vector.tensor_sub(out=dt[:, :, 0, 0:nt], in0=E0, in1=E1)
                if f < 5:
                    # d1 = O + E' carries the frame total: sum over all
                    # blocks of (O[r]+E[r+1]) telescopes to sum(x) per ci
                    for ci_t in range(2):
                        nc.vector.scalar_tensor_tensor(
                            out=dt[:, ci_t, 1, 0:nt],
                            in0=srv[:, ci_t, 1, rb:rb + nt, :], scalar=1.0,
                            in1=srv[:, ci_t, 0, rb + 1:rb + nt + 1, :],
                            op0=OP.mult, op1=OP.add,
                            accum_out=acc_tot[:, ci_t, b, f:f + 1])
                else:
                    nc.vector.tensor_add(out=dt[:, :, 1, 0:nt], in0=O0, in1=E1)
                nc.vector.tensor_sub(out=dt[:, :, 2, 0:nt], in0=E1, in1=O0)
                nc.vector.tensor_sub(out=dt[:, :, 3, 0:nt], in0=O0, in1=O1)

            raw_tiles = {}

            def emit_conv_pair(f, co_t, pair):
                """blocks 2*pair, 2*pair+1: matmuls + PSUM->SBUF copy per
                block, then one merged inverse over the pair"""
                raw = raw_tiles[f]
                cp = cppool.tile([128, 4, 896], BF16, tag="cp", name="cp")
                off = 0
                for b in (2 * pair, 2 * pair + 1):
                    r0, nt = BLK[b]
                    W = nt * 56
                    dt = dt_t[(f, b)]
                    m = mpsum.tile([128, 4, 512], F32, tag="m", name="m")
                    for j in range(4):
                        idx = 0
                        for ci_t in range(2):
                            for dx in range(3):
                                nc.tensor.matmul(
                                    m[:, j, 0:W],
                                    gw_sb[:, gidx(ci_t, j, dx, co_t)],
                                    dt[:, ci_t, j, 0:nt, dx:dx + 56],
                                    start=(idx == 0), stop=(idx == 5))
                                idx += 1
                    nc.scalar.activation(
                        out=cp[:, :, off:off + W], in_=m[:, :, 0:W], func=AF.Copy)
                    off += W
                r0, _ = BLK[2 * pair]
                ntp = (BLK[2 * pair][1] + BLK[2 * pair + 1][1])
                WP = ntp * 56
                re = raw[:, co_t, 0, r0:r0 + ntp, :]
                ro = raw[:, co_t, 1, r0:r0 + ntp, :]
                nc.vector.tensor_add(out=re, in0=cp[:, 0, 0:WP], in1=cp[:, 1, 0:WP])
                nc.vector.tensor_add(out=re, in0=re, in1=cp[:, 2, 0:WP])
                nc.vector.tensor_sub(out=ro, in0=cp[:, 1, 0:WP], in1=cp[:, 2, 0:WP])
                nc.vector.tensor_sub(out=ro, in0=ro, in1=cp[:, 3, 0:WP])

            def emit_transforms(f):
                for b in range(NB):
                    emit_transform(f, b)

            def emit_blocks(f, ph2=None):
                raw_tiles[f] = rawpool.tile([128, 2, 2, NT, 56], BF16,
                                            tag="raw", name="raw")
                for pair in range(2):
                    for co_t in range(2):
                        emit_conv_pair(f, co_t, pair)
                    if ph2 is not None:
                        emit_phase2(ph2, cos=(pair,))

            # ---------------- small ops (emitted after conv(1)) ----------
            def emit_small():
                # frame totals for f<5 come from the transform accum_outs
                for f in range(5):
                    nc.vector.tensor_reduce(
                        out=st_tot[:, :, f], in_=acc_tot[:, :, :, f],
                        axis=AX.X, op=OP.add)
                # window sums S[dy,dx] = total + a[dy] + b[dx] + corner
                a0 = spool.tile([128, 2, T], F32)
                a2 = spool.tile([128, 2, T], F32)
                nc.vector.tensor_sub(out=a0[:], in0=st_tot[:], in1=st_r56[:])
                nc.vector.tensor_sub(out=a2[:], in0=st_tot[:], in1=st_r1[:])
                S_all = spool.tile([128, 2, 9, T], F32)
                tmp = spool.tile([128, 2, T], F32)
                nc.vector.tensor_sub(out=tmp[:], in0=a0[:], in1=st_c56[:])
                nc.vector.tensor_add(out=S_all[:, :, 0], in0=tmp[:], in1=st_cB[:, :, 1])
                nc.vector.tensor_copy(out=S_all[:, :, 1], in_=a0[:])
                nc.vector.tensor_sub(out=tmp[:], in0=a0[:], in1=st_c1[:])
                nc.vector.tensor_add(out=S_all[:, :, 2], in0=tmp[:], in1=st_cB[:, :, 0])
                nc.vector.tensor_sub(out=S_all[:, :, 3], in0=st_tot[:], in1=st_c56[:])
                nc.vector.tensor_copy(out=S_all[:, :, 4], in_=st_tot[:])
                nc.vector.tensor_sub(out=S_all[:, :, 5], in0=st_tot[:], in1=st_c1[:])
                nc.vector.tensor_sub(out=tmp[:], in0=a2[:], in1=st_c56[:])
                nc.vector.tensor_add(out=S_all[:, :, 6], in0=tmp[:], in1=st_cA[:, :, 1])
                nc.vector.tensor_copy(out=S_all[:, :, 7], in_=a2[:])
                nc.vector.tensor_sub(out=tmp[:], in0=a2[:], in1=st_c1[:])
                nc.vector.tensor_add(out=S_all[:, :, 8], in0=tmp[:], in1=st_cA[:, :, 0])
                # B-transform over dy so the pooled matmul can reuse gw:
                # St[j=0] = S[dy0]-S[dy2]; St[1] = S[dy1]+S[dy2]; St[2] = S[dy2]-S[dy1]
                St = spool.tile([128, 2, 9, T], F32)
                nc.vector.tensor_sub(
                    out=St[:, :, 0:3], in0=S_all[:, :, 0:3], in1=S_all[:, :, 6:9])
                nc.vector.tensor_add(
                    out=St[:, :, 3:6], in0=S_all[:, :, 3:6], in1=S_all[:, :, 6:9])
                nc.vector.tensor_sub(
                    out=St[:, :, 6:9], in0=S_all[:, :, 6:9], in1=S_all[:, :, 3:6])
                St_bf = spool.tile([128, 2, 9, T], BF16)
                nc.vector.tensor_copy(out=St_bf[:], in_=St[:])
                pooled_sum = spool.tile([128, 2, T], F32)
                for co_t in range(2):
                    pm = mpsum.tile([128, T], F32, tag="m", name="pm")
                    idx = 0
                    for ci_t in range(2):
                        for j in range(3):
                            for dx in range(3):
                                nc.tensor.matmul(
                                    pm[:], gw_sb[:, gidx(ci_t, j, dx, co_t)],
                                    St_bf[:, ci_t, 3 * j + dx],
                                    start=(idx == 0), stop=(idx == 17))
                                idx += 1
                    nc.vector.tensor_copy(out=pooled_sum[:, co_t], in_=pm[:])
                total = spool.tile([128, 2], F32)
                nc.vector.tensor_reduce(
                    out=total[:], in_=pooled_sum[:], axis=AX.X, op=OP.add)
                xgpre = spool.tile([128, 2], BF16)
                for t in range(2):
                    nc.vector.tensor_scalar(
                        out=xgpre[:, t:t + 1], in0=total[:, t:t + 1],
                        scalar1=1.0 / (T * 3136.0), scalar2=netb_sb[:, t:t + 1],
                        op0=OP.mult, op1=OP.add)
                xg_ps = mpsum.tile([128, 2], F32, tag="m", name="xg_ps")
                for ct_ in range(2):
                    for kt in range(2):
                        nc.tensor.matmul(
                            xg_ps[:, ct_:ct_ + 1], lamw_sb[:, kt * 2 + ct_],
                            xgpre[:, kt:kt + 1], start=(kt == 0), stop=(kt == 1))
                xg = spool.tile([128, 2], F32)
                for t in range(2):
                    nc.scalar.activation(
                        out=xg[:, t:t + 1], in_=xg_ps[:, t:t + 1], func=AF.Identity,
                        bias=lamb_sb[:, t:t + 1])
                bxg = spool.tile([128, 2], F32)
                nc.vector.tensor_add(out=bxg[:], in0=netb_sb[:], in1=xg[:])
                pooled = spool.tile([128, 2, T], F32)
                for t in range(2):
                    nc.vector.tensor_scalar(
                        out=pooled[:, t], in0=pooled_sum[:, t],
                        scalar1=1.0 / 3136.0, scalar2=bxg[:, t:t + 1],
                        op0=OP.mult, op1=OP.add)
                pooled_bf = spool.tile([128, 2, T], BF16)
                nc.vector.tensor_copy(out=pooled_bf[:], in_=pooled[:])
                pT_ps = mpsum.tile([8, 256], BF16, tag="m", name="pT_ps")
                for t in range(2):
                    nc.tensor.transpose(
                        pT_ps[:, t * 128:(t + 1) * 128], pooled_bf[:, t], id_sb[:])
                pooledT = spool.tile([8, 256], BF16)
                nc.vector.tensor_copy(out=pooledT[:], in_=pT_ps[:])
                hdn_ps = mpsum.tile([16, 256], F32, tag="m", name="hdn_ps")
                nc.tensor.matmul(hdn_ps[:], w1t_sb[:], pooledT[:], start=True, stop=True)
                hdnr = spool.tile([16, 256], BF16)
                nc.scalar.activation(
                    out=hdnr[:], in_=hdn_ps[:], func=AF.Relu,
                    scale=bns_sb[:, 0:1], bias=bnsh_sb[:, 0:1])
                lgT_ps = mpsum.tile([3, 256], F32, tag="m", name="lgT_ps")
                nc.tensor.matmul(lgT_ps[:], w2t_sb[:], hdnr[:], start=True, stop=True)
                lgT = spool.tile([3, 256], BF16)
                nc.vector.tensor_copy(out=lgT[:], in_=lgT_ps[:])
                ew = spool.tile([128, 2, 3], F32)
                for t in range(2):
                    lg_ps = mpsum.tile([128, 3], BF16, tag="m", name="lg_ps")
                    nc.tensor.transpose(
                        lg_ps[:], lgT[:, t * 128:(t + 1) * 128], id_sb[0:3, 0:3])
                    nc.scalar.activation(out=ew[:, t], in_=lg_ps[:], func=AF.Exp)
                es = spool.tile([128, 2], F32)
                nc.vector.tensor_reduce(
                    out=es[:], in_=ew[:], axis=AX.X, op=OP.add)
                esr = spool.tile([128, 2], F32)
                nc.vector.reciprocal(out=esr[:], in_=es[:])
                wgt = spool.tile([128, 2, 3], F32)
                for t in range(2):
                    nc.vector.tensor_scalar_mul(
                        out=wgt[:, t], in0=ew[:, t], scalar1=esr[:, t:t + 1])
                # m[c,f] = mean_hw(lam_out) = temporal conv of pooled with wgt
                m = spool.tile([128, 2, T], F32)
                for t in range(2):
                    nc.vector.tensor_scalar_mul(
                        out=m[:, t], in0=pooled[:, t], scalar1=wgt[:, t, 1:2])
                    nc.vector.scalar_tensor_tensor(
                        out=m[:, t, 1:T], in0=pooled[:, t, 0:T - 1],
                        scalar=wgt[:, t, 0:1], in1=m[:, t, 1:T],
                        op0=OP.mult, op1=OP.add)
                    nc.vector.scalar_tensor_tensor(
                        out=m[:, t, 0:T - 1], in0=pooled[:, t, 1:T],
                        scalar=wgt[:, t, 2:3], in1=m[:, t, 0:T - 1],
                        op0=OP.mult, op1=OP.add)
                y = spool.tile([128, 2, T], F32)
                nc.vector.memset(y[:], 0.0)
                for t in range(2):
                    nc.vector.tensor_sub(
                        out=y[:, t, 0:T - 1], in0=m[:, t, 1:T], in1=m[:, t, 0:T - 1])
                y_bf = spool.tile([128, 2, T], BF16)
                nc.vector.tensor_copy(out=y_bf[:], in_=y[:])
                yT_ps = mpsum.tile([8, 256], BF16, tag="m", name="yT_ps")
                for t in range(2):
                    nc.tensor.transpose(
                        yT_ps[:, t * 128:(t + 1) * 128], y_bf[:, t], id_sb[:])
                yT = spool.tile([8, 256], F32)
                nc.vector.tensor_copy(out=yT[:], in_=yT_ps[:])
                ycT = spool.tile([8, 256], F32)
                nc.vector.tensor_scalar_mul(out=ycT[:], in0=yT[:], scalar1=float(me[1]))
                nc.vector.scalar_tensor_tensor(
                    out=ycT[:, 1:256], in0=yT[:, 0:255], scalar=float(me[0]),
                    in1=ycT[:, 1:256], op0=OP.mult, op1=OP.add)
                nc.vector.scalar_tensor_tensor(
                    out=ycT[:, 0:255], in0=yT[:, 1:256], scalar=float(me[2]),
                    in1=ycT[:, 0:255], op0=OP.mult, op1=OP.add)
                gateT = spool.tile([8, 256], BF16)
                nc.scalar.activation(out=gateT[:], in_=ycT[:], func=AF.Sigmoid)
                gate_c = spool.tile([128, 2, T], F32)
                for t in range(2):
                    g_ps = mpsum.tile([128, 8], BF16, tag="m", name="g_ps")
                    nc.tensor.transpose(
                        g_ps[:], gateT[:, t * 128:(t + 1) * 128], id_sb[0:8, 0:8])
                    nc.vector.tensor_copy(out=gate_c[:, t], in_=g_ps[:])
                # per-(c,f) scalars for phase 2
                g0 = spool.tile([128, 2, T], F32)
                g1 = spool.tile([128, 2, T], F32)
                g2 = spool.tile([128, 2, T], F32)
                g = [g0, g1, g2]
                for k in range(3):
                    for t in range(2):
                        nc.vector.tensor_scalar_mul(
                            out=g[k][:, t], in0=gate_c[:, t], scalar1=wgt[:, t, k:k + 1])
                goffs = spool.tile([128, 2, T], F32)
                w01 = spool.tile([128, 2], F32)
                w12 = spool.tile([128, 2], F32)
                for t in range(2):
                    nc.vector.tensor_scalar_mul(
                        out=goffs[:, t], in0=gate_c[:, t], scalar1=bxg[:, t:t + 1])
                    nc.vector.tensor_add(
                        out=w12[:, t:t + 1], in0=wgt[:, t, 1:2], in1=wgt[:, t, 2:3])
                    nc.vector.tensor_add(
                        out=w01[:, t:t + 1], in0=wgt[:, t, 0:1], in1=wgt[:, t, 1:2])
                    nc.vector.tensor_mul(
                        out=goffs[:, t, 0:1], in0=goffs[:, t, 0:1], in1=w12[:, t:t + 1])
                    nc.vector.tensor_mul(
                        out=goffs[:, t, 7:8], in0=goffs[:, t, 7:8], in1=w01[:, t:t + 1])
                return g0, g1, g2, goffs

            # ---------------- phase 2 (per frame) -------------------------
            W2 = NT * 56  # 1568, one even/odd plane
            gref = {}

            def emit_phase2(f, cos=(0, 1), on_gps=False):
                g0, g1, g2, goffs = gref['g']
                g = [g0, g1, g2]
                for co_t in cos:
                    fin = fpool.tile([128, 56, 56], F32, tag="fin", name="fin")
                    for eo in range(2):

                        def o(ff):
                            return raw_tiles[ff][:, co_t, eo]
                        fv = fin[:, eo::2, :]
                        A = wpool.tile([128, NT, 56], BF16, tag="A", name="A")

                        def final_tap(ftap, kg):
                            if on_gps:
                                Bp = wpool.tile([128, NT, 56], BF16, tag="Bp",
                                                name="Bp")
                                nc.scalar.mul(Bp[:], o(ftap), kg[:, co_t, f:f + 1])
                                nc.gpsimd.tensor_add(out=fv, in0=Bp[:], in1=A[:])
                            else:
                                nc.vector.scalar_tensor_tensor(
                                    out=fv, in0=o(ftap),
                                    scalar=kg[:, co_t, f:f + 1],
                                    in1=A[:], op0=OP.mult, op1=OP.add)
                        if f == 0 or f == T - 1:
                            fa, ka, fb, kb = (0, 1, 1, 2) if f == 0 else (T - 2, 0, T - 1, 1)
                            nc.scalar.activation(
                                out=A[:], in_=o(fa), func=AF.Identity,
                                scale=g[ka][:, co_t, f:f + 1], bias=goffs[:, co_t, f:f + 1])
                            final_tap(fb, g[kb])
                        else:
                            nc.scalar.activation(
                                out=A[:], in_=o(f - 1), func=AF.Identity,
                                scale=g0[:, co_t, f:f + 1], bias=goffs[:, co_t, f:f + 1])
                            Bp = wpool.tile([128, NT, 56], BF16, tag="Bp",
                                            name="Bp")
                            nc.scalar.mul(Bp[:], o(f), g1[:, co_t, f:f + 1])
                            nc.vector.tensor_add(out=A[:], in0=A[:], in1=Bp[:])
                            final_tap(f + 1, g2)
                    nc.sync.dma_start(
                        out=out_d.ap()[f, co_t * 128:(co_t + 1) * 128],
                        in_=fin[:])

            # ---------------- schedule ------------------------------------
            # Emission order = per-engine FIFO order, so it is chosen to keep
            # every op's gates pointing at earlier-emitted work: stats early
            # (they unblock the gate chain), transforms ~2 frames ahead of
            # their conv (dt ring bufs=6), phase 2 lagging 2 frames (raw ring
            # bufs=4 makes conv(f) wait on phase2(f-3) readers).
            emit_transforms(0)
            emit_stats(0)
            emit_blocks(0)
            emit_xin_dma(1)
            emit_transforms(1)
            emit_stats(1)
            emit_blocks(1)
            for f in range(2, 6):
                emit_xin_dma(f)
                emit_stats(f)
            emit_transforms(2)
            emit_blocks(2)
            emit_xin_dma(6)
            emit_stats(6)
            emit_transforms(3)
            emit_blocks(3)
            emit_xin_dma(7)
            emit_stats(7)
            emit_transforms(4)
            gref['g'] = emit_small()
            emit_phase2(0)
            emit_phase2(1)
            emit_blocks(4, ph2=2)
            emit_transforms(5)
            emit_blocks(5, ph2=3)
            emit_transforms(6)
            emit_blocks(6, ph2=4)
            emit_transforms(7)
            emit_phase2(5)
            emit_blocks(7)
            emit_phase2(6)
            emit_phase2(7)

    nc.compile()
    return nc


def _prep(inputs):
    x = np.asarray(inputs["x"], np.float32)          # (64,256,56,56)
    net_w = np.asarray(inputs["net_w"], np.float32)  # (256,256,3,3)
    net_b = np.asarray(inputs["net_b"], np.float32)
    lam_w = np.asarray(inputs["lam_w"], np.float32)
    lam_b = np.asarray(inputs["lam_b"], np.float32)
    mlp_w1 = np.asarray(inputs["mlp_w1"], np.float32)  # (16,8)
    mlp_w2 = np.asarray(inputs["mlp_w2"], np.float32)  # (3,16)
    bn_g = np.asarray(inputs["bn_gamma"], np.float32)
    bn_b = np.asarray(inputs["bn_beta"], np.float32)
    bn_m = np.asarray(inputs["bn_mean"], np.float32)
    bn_v = np.asarray(inputs["bn_var"], np.float32)
    me_w = np.asarray(inputs["me_w"], np.float32)

    bf = ml_dtypes.bfloat16
    xs = x.reshape(NCORES, T, 2, 128, 56, 56)
    xpad = np.zeros((NCORES, T, 2, 128, HP, HP), dtype=bf)
    xpad[:, :, :, :, 1:57, 1:57] = xs.astype(bf)
    # even/odd row planes: [..., 2, 29, 58]
    xeo = np.stack([xpad[:, :, :, :, 0::2, :], xpad[:, :, :, :, 1::2, :]], axis=4)
    xeo = np.ascontiguousarray(xeo.reshape(NCORES, T, 2, 128, PADSZ))

    # G-transformed (over dy) weights, one 128x128 chunk per (ci_t,j,dx,co_t)
    G = np.array([[1, 0, 0], [.5, .5, .5], [.5, -.5, .5], [0, 0, 1]], np.float32)
    gw_full = np.einsum('jy,oiyx->oijx', G, net_w)       # (256,256,4,3)
    gw = gw_full.reshape(2, 128, 2, 128, 4, 3).transpose(2, 4, 5, 0, 3, 1)
    gw = np.ascontiguousarray(gw.reshape(48, 128, 128).astype(bf))

    lamw = lam_w.T.reshape(2, 128, 2, 128).transpose(0, 2, 1, 3)
    lamw = np.ascontiguousarray(lamw.reshape(4, 128, 128).astype(bf))
    w1t = np.ascontiguousarray(mlp_w1.T.astype(bf))      # (8,16)
    w2t = np.ascontiguousarray(mlp_w2.T.astype(bf))      # (16,3)
    bns = (bn_g / np.sqrt(bn_v + 1e-5)).astype(np.float32).reshape(16, 1)
    bnsh = (bn_b - bn_m * bns[:, 0]).astype(np.float32).reshape(16, 1)
    netb = np.ascontiguousarray(net_b.reshape(2, 128))
    lamb = np.ascontiguousarray(lam_b.reshape(2, 128))
    ident = np.eye(128, dtype=bf)

    common = dict(gw=gw, lamw=lamw, w1t=w1t, w2t=w2t, bns=bns, bnsh=bnsh,
                  netb=netb, lamb=lamb, ident=ident)
    in_maps = [dict(x=xeo[i], **common) for i in range(NCORES)]
    return in_maps, tuple(float(v) for v in me_w)


def kernel(**inputs):
    in_maps, me = _prep(inputs)
    nc = _CACHE.get(me)
    if nc is None:
        nc = _build(me)
        _CACHE[me] = nc
    res = run_bass_kernel_spmd(nc, in_maps, core_ids=list(range(NCORES)))
    out = np.stack([res.results[i]["out"] for i in range(NCORES)])  # (8,8,256,3136)
    return np.ascontiguousarray(out.reshape(64, 256, 56, 56))
